# revision 44
# baseline (speedup 1.0000x reference)
"""Trainium2 Bass kernel for DifferentialAttention (B=2, S=2048, DIM=2048).

Sharding: 8 cores = 2 batches x 4 head-groups (4 heads each). Per core:
  - QKV projection (column-parallel slices of wq/wk/wv) + RoPE on device
  - differential attention for its 4 heads
  - row-parallel wo partial product; host sums the 4 partials per batch.

v2 design (cost-model driven):
  * scores in [q, k] layout: psc[128 q, 1024 k] per (h, branch, qtile).
    exp on Act engine with accum_out -> per-query denominators for FREE
    (no M=1 denominator matmuls on PE: saves ~55us PE per core).
  * et [q, k] transposed to [k, q] chunks via DMA-transpose (XBAR, 14ns
    per 16x128 tile, off-engine) for the pv matmuls.
  * pv out [q 128, dv 128]: per-query quantities (d1, d2, rr, rs) are
    per-PARTITION columns -> stage C uses tensor_scalar per-partition
    scalar APs; no broadcast matmuls, no row DMAs.
  * u' = lam*(d1/d2)*pv2 - pv1 = -u; the sign is folded into a host-side
    negation of wo. rsqrt via Quake bit-trick + 2 Newton steps on DVE
    (keeps Act on {Copy, Exp} = one act table set, no table reloads).
  * wo needs attn^T: small DMA-transpose per qtile, then 4x4 matmuls.
  * RoPE: head-dim perm puts rope partners 16 rows apart inside each
    32-partition quadrant, so the cross-partner swap is a single DVE
    stream_shuffle (no DMA, no extra matmul); the cos/sin multiplies run
    on the otherwise-idle GPSIMD engine.
  * B/C software-pipelined with skew (scores qt | pv qt-2 | wo qt-3),
    interleaved at (h, j)-beat granularity so the in-order PE queue
    never head-blocks on Act/DMA results; the s-half-1 V and Q
    projections are woven into the first three steps' beats (their psum
    borrows the banks the wo stage does not need until step 3).
  * out DMAs, woven weight loads and wot go through the GPSIMD (SWDGE)
    queue so the SP sequencer carries only the XBAR transposes.

Per-core layouts (partition dim first):
  QT [128,4,S]: q heads; rows = [branch j | quadrant q | E(16) O(16)],
     row j*64+q*32+c*16+i <-> head-dim 2*(16q+i)+c.
  KT [128,2,S]: same for the 2 kv heads.
  Vn [128,16,256]: v natural [s, dv] layout, s-tile major.
  et [128 q, 8 hj, 1024 k]; etT [128 k, 64, 128 q] via DMA transpose.
"""

import math
import numpy as np
import ml_dtypes
import concourse.bass as bass
import concourse.tile as tile
from concourse import bacc, mybir
from concourse.bass_utils import run_bass_kernel_spmd
from contextlib import ExitStack

F32 = mybir.dt.float32
BF16 = mybir.dt.bfloat16
AF = mybir.ActivationFunctionType
ALU = mybir.AluOpType

DIM = 2048
S = 2048
B = 2
HD = 64          # rope head dim
EPS = 1e-5
SCALE = HD ** -0.5
NCORES = 8
NQT = S // 128   # 16 query tiles

TRACE = False    # set by test.py to collect an NTFF profile
LAST_RESULTS = None

# stream_shuffle mask: swap 16-row halves within each 32-partition quadrant
_SWAP16 = list(range(16, 32)) + list(range(0, 16))


# ---------------------------------------------------------------- device program

def build_program(lam: float):
    nc = bacc.Bacc("TRN2", target_bir_lowering=False, debug=False,
                   num_devices=NCORES)
    io = {}
    for name, shape, d in [
        ("xT", [DIM, S], BF16), ("wq_s", [DIM, 512], BF16),
        ("wk_s", [DIM, 256], BF16), ("wv_s", [DIM, 256], BF16),
        ("wo_s", [512, DIM], BF16),
        ("cs128", [128, S], BF16), ("sn128", [128, S], BF16),
    ]:
        io[name] = nc.dram_tensor(name, shape, d, kind="ExternalInput").ap()
    out = nc.dram_tensor("out", [S, DIM], BF16, kind="ExternalOutput").ap()

    with tile.TileContext(nc) as tc:
        _body(tc, io, out, lam)
    nc.compile()
    return nc


def _body(tc, io, out, lam):
    nc = tc.nc
    with ExitStack() as top:
        persist = top.enter_context(tc.tile_pool(name="persist", bufs=1))
        QT = persist.tile([128, 4, S], BF16)
        KT = persist.tile([128, 2, S], BF16)
        Vn = persist.tile([128, 16, 256], BF16)

        # B pools first (bottom of the pool stacks) so the A pools can
        # be released mid-B in LIFO order
        bctx = ExitStack()
        b = _stage_bc_pools(tc, bctx)
        actx = ExitStack()
        apsum = ExitStack()
        a = _stage_a_setup(tc, actx, apsum, io)
        # A emission: kv+q of half 0, kv of half 1. q(half 1) is woven
        # into the first B steps (B only needs K/V complete + Q half 0).
        _emit_xh(tc, a, 0)
        _emit_k(tc, a, 0, KT)
        _emit_v(tc, a, 0, Vn)
        _emit_xh(tc, a, 1)
        _emit_q(tc, a, 0, QT, KT, Vn, act_evac=True)
        a["xh"] = a["xh_next"]
        _emit_k(tc, a, 1, KT)
        # free A's x buffer and wide psum pools; the woven q(1)
        # projection streams x chunks and gets a single-buffer psum pool
        apsum.close()
        b["pss"] = bctx.enter_context(tc.tile_pool(name="pss", bufs=2,
                                                   space="PSUM"))
        b["ppvp"] = bctx.enter_context(tc.tile_pool(name="ppv", bufs=1,
                                                    space="PSUM"))
        a["ps"] = actx.enter_context(tc.tile_pool(name="pq1", bufs=1,
                                                  space="PSUM"))
        with bctx:
            _stage_bc(tc, io, out, QT, KT, Vn, lam, a, actx, b)


def _stage_a_setup(tc, ctx, psctx, io):
    nc = tc.nc
    a = {}
    a["wp"] = ctx.enter_context(tc.tile_pool(name="wqk", bufs=2))
    a["trig"] = ctx.enter_context(tc.tile_pool(name="trig", bufs=1))
    a["tmp"] = ctx.enter_context(tc.tile_pool(name="ropetmp", bufs=1))
    a["xp"] = ctx.enter_context(tc.tile_pool(name="xh", bufs=2))
    a["wvp"] = ctx.enter_context(tc.tile_pool(name="wvp", bufs=1))
    a["ps"] = psctx.enter_context(tc.tile_pool(name="ps_qk", bufs=2,
                                               space="PSUM"))
    a["psv"] = psctx.enter_context(tc.tile_pool(name="ps_v", bufs=2,
                                                space="PSUM"))
    a["xT3"] = io["xT"].rearrange("(a p) s -> p a s", p=128)
    a["wq3"] = io["wq_s"].rearrange("(a p) c -> p a c", p=128)
    a["wk3"] = io["wk_s"].rearrange("(a p) c -> p a c", p=128)

    # first x half chunked so the first matmuls can start early
    xh0 = a["xp"].tile([128, 16, 1024], BF16, tag="xh", name="xh0")
    nc.sync.dma_start(xh0[:, 0:2, :], a["xT3"][:, 0:2, 0:1024])
    wk00 = a["wp"].tile([128, 16, 128], BF16, tag="w", name="wk00")
    nc.sync.dma_start(wk00[:, 0:4, :], a["wk3"][:, 0:4, 0:128])
    nc.sync.dma_start(xh0[:, 2:4, :], a["xT3"][:, 2:4, 0:1024])
    nc.sync.dma_start(wk00[:, 4:16, :], a["wk3"][:, 4:16, 0:128])
    for c in range(1, 4):
        nc.sync.dma_start(xh0[:, c * 4:(c + 1) * 4, :],
                          a["xT3"][:, c * 4:(c + 1) * 4, 0:1024])
    a["xh0"] = xh0
    a["wk00"] = wk00

    cs = a["trig"].tile([128, S], BF16)
    sn = a["trig"].tile([128, S], BF16)
    nc.sync.dma_start(cs[:], io["cs128"][:])
    nc.sync.dma_start(sn[:], io["sn128"][:])
    a["cs"], a["sn"] = cs, sn
    wv_all = a["wvp"].tile([128, 16, 256], BF16)
    nc.sync.dma_start(wv_all[:],
                      io["wv_s"].rearrange("(a p) c -> p a c", p=128))
    a["wv_all"] = wv_all
    return a


def _emit_xh(tc, a, sq):
    if sq == 0:
        a["xh"] = a["xh0"]
        return
    nc = tc.nc
    xh = a["xp"].tile([128, 16, 1024], BF16, tag="xh", name=f"xh{sq}")
    nc.sync.dma_start(xh[:], a["xT3"][:, :, sq * 1024:(sq + 1) * 1024])
    a["xh_next"] = xh


def _rope(tc, a, pq, dest_ap, ssl, act_evac):
    """c0 = evac(pq); dest = c0*cs + shuffle(c0*sn)."""
    nc = tc.nc
    W = 1024
    c0 = a["tmp"].tile([128, W], BF16, tag="c0", name="c0")
    if act_evac:
        nc.scalar.activation(c0[:], pq[:], AF.Copy, bias=0.0, scale=1.0)
    else:
        # during the B overlap the Act engine is saturated with exps
        nc.vector.tensor_copy(c0[:], pq[:])
    t2 = a["tmp"].tile([128, W], BF16, tag="t2", name="t2")
    t2s = a["tmp"].tile([128, W], BF16, tag="t2s", name="t2s")
    nc.gpsimd.tensor_mul(t2[:], c0[:], a["sn"][:, ssl])
    nc.vector.stream_shuffle(t2s[:], t2[:], _SWAP16)
    nc.gpsimd.tensor_mul(c0[:], c0[:], a["cs"][:, ssl])
    nc.vector.tensor_add(dest_ap, c0[:], t2s[:])


def _emit_k(tc, a, sq, KT):
    """K projection (2 kv tiles) + rope for s-half sq."""
    nc = tc.nc
    W = 1024
    ssl = slice(sq * W, sq * W + W)
    xh = a["xh"]
    for kt_i in range(2):
        if sq == 0 and kt_i == 0:
            wct = a["wk00"]
        else:
            wct = a["wp"].tile([128, 16, 128], BF16, tag="w",
                               name=f"wk{sq}{kt_i}")
            nc.sync.dma_start(wct[:],
                              a["wk3"][:, :, kt_i * 128:(kt_i + 1) * 128])
        pq = a["ps"].tile([128, W], F32, tag="psqk", name="pqk")
        for dt_i in range(16):
            for nch in range(2):
                nsl = slice(nch * 512, (nch + 1) * 512)
                nc.tensor.matmul(pq[:, nsl], lhsT=wct[:, dt_i, :],
                                 rhs=xh[:, dt_i, nsl],
                                 start=(dt_i == 0), stop=(dt_i == 15))
        _rope(tc, a, pq, KT[:, kt_i, ssl], ssl, act_evac=True)


def _v_st_tasks(tc, a, sq, st, Vn, inline):
    """Closures for one V s-tile. In the woven (sq=1) path the psum is a
    256-column sub-slice of the shared single-buffer projection tile."""
    nc = tc.nc
    xh = a["xh"]
    state = {}

    def mms(lo, hi):
        def run():
            if lo == 0:
                if inline:
                    pv = a["psv"].tile([128, 256], F32, tag="psv",
                                       name="psv")
                else:
                    pvt = a["ps"].tile([128, 1024], F32, tag="psqk",
                                       name="psv1")
                    pv = pvt[:, 0:256]
                state["pv"] = pv
            for dt_i in range(lo, hi):
                nc.tensor.matmul(state["pv"],
                                 lhsT=xh[:, dt_i, st * 128:(st + 1) * 128],
                                 rhs=a["wv_all"][:, dt_i, :],
                                 start=(dt_i == 0), stop=(dt_i == 15))
        return run

    def copy():
        nc.vector.tensor_copy(Vn[:, sq * 8 + st, :], state["pv"])

    return [mms(0, 8), mms(8, 16), copy]


def _emit_v(tc, a, sq, Vn):
    for st in range(8):
        for task in _v_st_tasks(tc, a, sq, st, Vn, inline=True):
            task()


def _emit_q(tc, a, sq, QT, KT, Vn, act_evac):
    for ct in range(4):
        for task in _q_ct_tasks(tc, a, sq, ct, QT, act_evac):
            task()


def _q_ct_tasks(tc, a, sq, ct, QT, act_evac):
    """Closures for one Q head-tile projection: 16 matmul steps + rope.

    When a["xh"] is None (the B-overlap weave), each matmul streams its
    own x chunk from DRAM through a small rotating pool."""
    nc = tc.nc
    W = 1024
    ssl = slice(sq * W, sq * W + W)
    state = {}

    def load():
        wct = a["wp"].tile([128, 16, 128], BF16, tag="w",
                           name=f"wq{sq}{ct}")
        dma = nc.sync.dma_start if sq == 0 else nc.gpsimd.dma_start
        dma(wct[:], a["wq3"][:, :, ct * 128:(ct + 1) * 128])
        pq = a["ps"].tile([128, W], F32, tag="psqk", name="pqq")
        state["wct"], state["pq"] = wct, pq

    def mm(dt_i):
        def run():
            xsrc = a["xh"][:, dt_i, :]
            for nch in range(2):
                nsl = slice(nch * 512, (nch + 1) * 512)
                nc.tensor.matmul(state["pq"][:, nsl],
                                 lhsT=state["wct"][:, dt_i, :],
                                 rhs=xsrc[:, nsl],
                                 start=(dt_i == 0), stop=(dt_i == 15))
        return run

    def rope():
        _rope(tc, a, state["pq"], QT[:, ct, ssl], ssl, act_evac)

    return [load] + [mm(i) for i in range(16)] + [rope]


def _stage_bc_pools(tc, ctx):
    nc = tc.nc
    b = {"ctx": ctx}
    b["etpA"] = ctx.enter_context(tc.tile_pool(name="etpA", bufs=1))
    b["etpB"] = ctx.enter_context(tc.tile_pool(name="etpB", bufs=1))
    b["etTp"] = ctx.enter_context(tc.tile_pool(name="etTp", bufs=2))
    b["dcp"] = ctx.enter_context(tc.tile_pool(name="dcp", bufs=3))
    b["colp"] = ctx.enter_context(tc.tile_pool(name="colp", bufs=2))
    b["up"] = ctx.enter_context(tc.tile_pool(name="up", bufs=2))
    b["atp"] = ctx.enter_context(tc.tile_pool(name="atp", bufs=2))
    b["atTp"] = ctx.enter_context(tc.tile_pool(name="atTp", bufs=2))
    magp = ctx.enter_context(tc.tile_pool(name="magic", bufs=1))
    b["magicT"] = magp.tile([128, 4], mybir.dt.uint32, name="magicT")
    nc.gpsimd.memset(b["magicT"][:], 0x5F3759DF)
    return b


def _stage_bc(tc, io, out, QT, KT, Vn, lam, a, actx, b):
    """Attention + norm + wo, software-pipelined over 16 query tiles.

    step s: scores+exp+transpose(qt=s) | pv+norm(qt=s-2) | wo+out(qt=s-3).
    The three PE workloads interleave at (h, j)-beat granularity so the
    in-order PE queue always has ready work while Act runs the exps.
    The Q projection of s-half 1 is woven into steps 0-2 (its PSUM use
    borrows the banks the wo stage does not need until step 3).
    """
    nc = tc.nc
    # v+q of s-half 1, interleaved into the early-step beats; each q
    # weight load is hoisted one head-tile ahead of its matmuls
    proj = []
    for st in range(8):
        proj.extend(_v_st_tasks(tc, a, 1, st, Vn, inline=False))
    ct_tasks = [_q_ct_tasks(tc, a, 1, ct, QT, act_evac=False)
                for ct in range(4)]
    proj += [ct_tasks[0][0], ct_tasks[1][0]] + ct_tasks[0][1:]
    for ct in range(1, 4):
        if ct + 1 < 4:
            proj.append(ct_tasks[ct + 1][0])
        proj.extend(ct_tasks[ct][1:])
    proj_i = [0]

    def drain_proj(n):
        k = 0
        while k < n and proj_i[0] < len(proj):
            proj[proj_i[0]]()
            proj_i[0] += 1
            k += 1

    ctx = b["ctx"]
    etTp, dcp, colp = b["etTp"], b["dcp"], b["colp"]
    up, atp, atTp = b["up"], b["atp"], b["atTp"]
    pss, ppvp = b["pss"], b["ppvp"]
    magicT = b["magicT"]
    outwp = None
    psop = None
    if True:

        etT_t = [None] * NQT
        dc_t = [None] * NQT
        atT_t = [None] * NQT

        for step in range(NQT + 3):
            fr = step if step < NQT else None
            mid = step - 2 if 2 <= step < NQT + 2 else None
            bk = step - 3 if 3 <= step else None

            if step == 3:
                # q(half 1) projection is done; its A pools (and PSUM
                # banks) make room for the wo accumulators
                assert proj_i[0] == len(proj)
                actx.close()
                psop = ctx.enter_context(tc.tile_pool(name="pso", bufs=2,
                                                      space="PSUM"))
                outwp = ctx.enter_context(tc.tile_pool(name="outwp", bufs=2))

                wotp = ctx.enter_context(tc.tile_pool(name="wotp", bufs=1))
                wot = wotp.tile([128, 4, S], BF16, name="wot")
                nc.gpsimd.dma_start(wot[:], io["wo_s"].rearrange(
                    "(a p) c -> p a c", p=128))

            if fr is not None:
                etpool = [b["etpA"], b["etpB"]][fr % 2]
                et = etpool.tile([128, 8, 1024], BF16, tag="et", name="et")
                etT = etTp.tile([128, 64, 128], BF16, tag="etT")
                etT_t[fr] = etT
                dc = dcp.tile([128, 8], F32, tag="dc")
                dc_t[fr] = dc
            if mid is not None:
                dcm = dc_t[mid]
                rec = colp.tile([128, 4], F32, tag="rec")
                rrl = colp.tile([128, 4], F32, tag="rrl")
                msum = colp.tile([128, 4], F32, tag="msum")
                tcol = colp.tile([128, 4], F32, tag="tcol")
                ya = colp.tile([128, 4], F32, tag="ya")
                aa = colp.tile([128, 4], F32, tag="aa")
                shu = colp.tile([128, 4], mybir.dt.uint32, tag="shu")
                nc.vector.reciprocal(rec[:], dcm[:, 4:8])
                nc.vector.scalar_tensor_tensor(
                    rrl[:], dcm[:, 0:4], float(lam), rec[:],
                    op0=ALU.mult, op1=ALU.mult)
                # two psum tiles (h0+h1 / h2+h3, slots [j0, j1] per h);
                # pv beats alternate tiles (h order 0,2,1,3) so a pv
                # write never WAR-waits on the previous head's u' reads
                ppvA = ppvp.tile([128, 4, 128], F32, tag="ppvA")
                ppvB = ppvp.tile([128, 4, 128], F32, tag="ppvB")
                def ppv_ap(h, j):
                    t = ppvA if h < 2 else ppvB
                    return t[:, (h % 2) * 2 + j, :]
                u = up.tile([128, 4, 128], BF16, tag="u")
                usq = up.tile([128, 128], BF16, tag="usq")
                v2 = up.tile([128, 4, 128], BF16, tag="v2")
                at = atp.tile([128, 4, 128], BF16, tag="at")
            if bk is not None:
                atTb = atT_t[bk]
                outw = outwp.tile([128, 2048], BF16, tag="outw")

            H_ORDER = (0, 2, 1, 3)
            for beat in range(8):
                if step < 3:
                    drain_proj(5)
                # frontend: one (h, j) scores pair + exp(+denominator)
                if fr is not None:
                    hj = beat
                    j, h = divmod(hj, 4)
                    kvl, rho = h // 2, h % 2
                    jsl = slice(j * 64, j * 64 + 64)
                    qsl = slice(fr * 128, fr * 128 + 128)
                    psc = pss.tile([128, 1024], F32, tag="sc")
                    for nch in range(2):
                        nsl = slice(nch * 512, (nch + 1) * 512)
                        nc.tensor.matmul(
                            psc[:, nsl],
                            lhsT=QT[jsl, h, qsl],
                            rhs=KT[jsl, kvl,
                                   rho * 1024 + nch * 512:
                                   rho * 1024 + nch * 512 + 512],
                            start=True, stop=True)
                    nc.scalar.activation(et[:, hj, :], psc[:], AF.Exp,
                                         bias=0.0, scale=float(SCALE),
                                         accum_out=dc[:, hj:hj + 1])
                # middle: pv accumulation for one (h, j)
                if mid is not None:
                    h = H_ORDER[beat // 2]
                    j = beat % 2
                    hj = j * 4 + h
                    kvl, rho = h // 2, h % 2
                    etTm = etT_t[mid]
                    pdst = ppv_ap(h, j)
                    for kt in range(8):
                        nc.tensor.matmul(
                            pdst,
                            lhsT=etTm[:, hj * 8 + kt, :],
                            rhs=Vn[:, rho * 8 + kt,
                                   kvl * 128:(kvl + 1) * 128],
                            start=(kt == 0), stop=(kt == 7))
                    # u' for head h once both branches are accumulated
                    if j == 1:
                        nc.vector.tensor_scalar_mul(
                            v2[:, h, :], ppv_ap(h, 1), rrl[:, h:h + 1])
                        nc.vector.tensor_sub(u[:, h, :], v2[:, h, :],
                                             ppv_ap(h, 0))
                        nc.vector.scalar_tensor_tensor(
                            usq[:], u[:, h, :], 1.0, u[:, h, :],
                            op0=ALU.mult, op1=ALU.mult,
                            accum_out=msum[:, h:h + 1])
                # backend: one e-chunk of the wo matmul
                if bk is not None and beat < 4:
                    ech = beat
                    pso = psop.tile([128, 512], F32, tag="pso")
                    for r in range(4):
                        nc.tensor.matmul(
                            pso[:], lhsT=atTb[:, r, :],
                            rhs=wot[:, r, ech * 512:(ech + 1) * 512],
                            start=(r == 0), stop=(r == 3))
                    nc.vector.tensor_copy(outw[:, ech * 512:(ech + 1) * 512],
                                          pso[:])

            if step < 3:
                drain_proj(len(proj))
            if mid is not None:
                # rs = rsqrt(eps*d1^2 + sum(u^2)/128), Quake bit-trick +
                # two Newton steps on the DVE (keeps Act on {Copy, Exp}
                # only -> a single activation-table set, no reloads)
                nc.vector.scalar_tensor_tensor(
                    tcol[:], dcm[:, 0:4], float(EPS), dcm[:, 0:4],
                    op0=ALU.mult, op1=ALU.mult)
                nc.vector.scalar_tensor_tensor(
                    tcol[:], msum[:], float(1.0 / 128.0), tcol[:],
                    op0=ALU.mult, op1=ALU.add)
                nc.vector.tensor_scalar(shu[:], tcol[:].bitcast(
                    mybir.dt.uint32), 1, None, op0=ALU.arith_shift_right)
                nc.vector.tensor_sub(ya[:].bitcast(mybir.dt.uint32),
                                     magicT[:], shu[:])
                for _ in range(2):
                    nc.vector.tensor_mul(aa[:], ya[:], ya[:])
                    nc.vector.tensor_mul(aa[:], aa[:], tcol[:])
                    nc.vector.tensor_scalar(aa[:], aa[:], -0.5, 1.5,
                                            op0=ALU.mult, op1=ALU.add)
                    nc.vector.tensor_mul(ya[:], ya[:], aa[:])
                for h in range(4):
                    nc.vector.tensor_scalar_mul(at[:, h, :], u[:, h, :],
                                                ya[:, h:h + 1])
                atT = atTp.tile([128, 4, 128], BF16, tag="atT")
                atT_t[mid] = atT
                nc.sync.dma_start_transpose(atT[:], at[:])

            if fr is not None:
                nc.sync.dma_start_transpose(etT[:], et[:])

            if bk is not None:
                # Pool-issued (SWDGE) so the out write never queues behind
                # the transposes on the SP sequencer
                nc.gpsimd.dma_start(out[bk * 128:(bk + 1) * 128, :], outw[:])


# ---------------------------------------------------------------- host side

# row (q*32 + c*16 + i) within a branch <-> head-dim 2*(16q+i)+c
_PERM64 = np.empty(64, np.int64)
for _q in range(2):
    for _c in range(2):
        for _i in range(16):
            _PERM64[_q * 32 + _c * 16 + _i] = 2 * (16 * _q + _i) + _c


def make_core_inputs(core, x, wq, wk, wv, wo, subln_w, lambda_init,
                     freqs_cos, freqs_sin):
    b, g = divmod(core, 4)
    npdt = ml_dtypes.bfloat16
    qcols = np.empty(512, np.int64)
    for hl in range(4):
        for j in range(2):
            qcols[hl * 128 + j * 64:hl * 128 + j * 64 + 64] = \
                ((4 * g + hl) * 2 + j) * 64 + _PERM64
    kcols = np.empty(256, np.int64)
    for kvl in range(2):
        for j in range(2):
            kcols[kvl * 128 + j * 64:kvl * 128 + j * 64 + 64] = \
                ((2 * g + kvl) * 2 + j) * 64 + _PERM64
    vcols = np.arange(256) + 2 * g * 128

    cosT = np.ascontiguousarray(freqs_cos.T.astype(np.float32))  # [32, S]
    sinT = np.ascontiguousarray(freqs_sin.T.astype(np.float32))
    cs64 = np.concatenate([cosT[0:16], cosT[0:16],
                           cosT[16:32], cosT[16:32]], axis=0)
    sn64 = np.concatenate([sinT[0:16], -sinT[0:16],
                           sinT[16:32], -sinT[16:32]], axis=0)
    # wo rows carry subln*(1-lambda_init) and the global sign flip (u' = -u)
    wo_s = wo[512 * g: 512 * g + 512, :].astype(np.float32).copy()
    wo_s *= -np.tile(subln_w.astype(np.float32)
                     * (1.0 - np.float32(np.asarray(lambda_init)[0])),
                     4)[:, None]
    return {
        "xT": np.ascontiguousarray(x[b].T.astype(np.float32)).astype(npdt),
        "wq_s": np.ascontiguousarray(wq[:, qcols].astype(np.float32)).astype(npdt),
        "wk_s": np.ascontiguousarray(wk[:, kcols].astype(np.float32)).astype(npdt),
        "wv_s": np.ascontiguousarray(wv[:, vcols].astype(np.float32)).astype(npdt),
        "wo_s": wo_s.astype(npdt),
        "cs128": np.tile(cs64, (2, 1)).astype(npdt),
        "sn128": np.tile(sn64, (2, 1)).astype(npdt),
    }


def compute_lambda(lambda_q1, lambda_k1, lambda_q2, lambda_k2, lambda_init):
    l1 = np.exp(np.sum(np.float32(lambda_q1) * np.float32(lambda_k1),
                       dtype=np.float32))
    l2 = np.exp(np.sum(np.float32(lambda_q2) * np.float32(lambda_k2),
                       dtype=np.float32))
    return float(l1 - l2 + np.float32(np.asarray(lambda_init)[0]))


def kernel(x, wq, wk, wv, wo, lambda_q1, lambda_k1, lambda_q2, lambda_k2,
           lambda_init, subln_w, freqs_cos, freqs_sin):
    global LAST_RESULTS
    x = np.asarray(x); wq = np.asarray(wq); wk = np.asarray(wk)
    wv = np.asarray(wv); wo = np.asarray(wo)
    lam = compute_lambda(lambda_q1, lambda_k1, lambda_q2, lambda_k2, lambda_init)

    nc = build_program(lam)
    in_maps = [make_core_inputs(c, x, wq, wk, wv, wo,
                                np.asarray(subln_w), np.asarray(lambda_init),
                                np.asarray(freqs_cos), np.asarray(freqs_sin))
               for c in range(NCORES)]
    res = run_bass_kernel_spmd(nc, in_maps, list(range(NCORES)), trace=TRACE)
    LAST_RESULTS = res
    outs = [res.results[c]["out"] for c in range(NCORES)]
    full = np.empty((B, S, DIM), np.float32)
    for b in range(B):
        full[b] = (outs[4 * b].astype(np.float32)
                   + outs[4 * b + 1].astype(np.float32)
                   + outs[4 * b + 2].astype(np.float32)
                   + outs[4 * b + 3].astype(np.float32))
    return full


# revision 46
# speedup vs baseline: 1.0001x; 1.0001x over previous
"""Trainium2 Bass kernel for DifferentialAttention (B=2, S=2048, DIM=2048).

Sharding: 8 cores = 2 batches x 4 head-groups (4 heads each). Per core:
  - QKV projection (column-parallel slices of wq/wk/wv) + RoPE on device
  - differential attention for its 4 heads
  - row-parallel wo partial product; host sums the 4 partials per batch.

v2 design (cost-model driven):
  * scores in [q, k] layout: psc[128 q, 1024 k] per (h, branch, qtile).
    exp on Act engine with accum_out -> per-query denominators for FREE
    (no M=1 denominator matmuls on PE: saves ~55us PE per core).
  * et [q, k] transposed to [k, q] chunks via DMA-transpose (XBAR, 14ns
    per 16x128 tile, off-engine) for the pv matmuls.
  * pv out [q 128, dv 128]: per-query quantities (d1, d2, rr, rs) are
    per-PARTITION columns -> stage C uses tensor_scalar per-partition
    scalar APs; no broadcast matmuls, no row DMAs.
  * u' = lam*(d1/d2)*pv2 - pv1 = -u; the sign is folded into a host-side
    negation of wo. rsqrt via Quake bit-trick + 2 Newton steps on DVE
    (keeps Act on {Copy, Exp} = one act table set, no table reloads).
  * wo needs attn^T: small DMA-transpose per qtile, then 4x4 matmuls.
  * RoPE: head-dim perm puts rope partners 16 rows apart inside each
    32-partition quadrant, so the cross-partner swap is a single DVE
    stream_shuffle (no DMA, no extra matmul); the cos/sin multiplies run
    on the otherwise-idle GPSIMD engine.
  * B/C software-pipelined with skew (scores qt | pv qt-2 | wo qt-3),
    interleaved at (h, j)-beat granularity so the in-order PE queue
    never head-blocks on Act/DMA results; the s-half-1 V and Q
    projections are woven into the first three steps' beats (their psum
    borrows the banks the wo stage does not need until step 3).
  * out DMAs, woven weight loads and wot go through the GPSIMD (SWDGE)
    queue so the SP sequencer carries only the XBAR transposes.

Per-core layouts (partition dim first):
  QT [128,4,S]: q heads; rows = [branch j | quadrant q | E(16) O(16)],
     row j*64+q*32+c*16+i <-> head-dim 2*(16q+i)+c.
  KT [128,2,S]: same for the 2 kv heads.
  Vn [128,16,256]: v natural [s, dv] layout, s-tile major.
  et [128 q, 8 hj, 1024 k]; etT [128 k, 64, 128 q] via DMA transpose.
"""

import math
import numpy as np
import ml_dtypes
import concourse.bass as bass
import concourse.tile as tile
from concourse import bacc, mybir
from concourse.bass_utils import run_bass_kernel_spmd
from contextlib import ExitStack

F32 = mybir.dt.float32
BF16 = mybir.dt.bfloat16
AF = mybir.ActivationFunctionType
ALU = mybir.AluOpType

DIM = 2048
S = 2048
B = 2
HD = 64          # rope head dim
EPS = 1e-5
SCALE = HD ** -0.5
NCORES = 8
NQT = S // 128   # 16 query tiles

TRACE = False    # set by test.py to collect an NTFF profile
LAST_RESULTS = None

# stream_shuffle mask: swap 16-row halves within each 32-partition quadrant
_SWAP16 = list(range(16, 32)) + list(range(0, 16))


# ---------------------------------------------------------------- device program

def build_program(lam: float):
    nc = bacc.Bacc("TRN2", target_bir_lowering=False, debug=False,
                   num_devices=NCORES)
    io = {}
    for name, shape, d in [
        ("xT", [DIM, S], BF16), ("wq_s", [DIM, 512], BF16),
        ("wk_s", [DIM, 256], BF16), ("wv_s", [DIM, 256], BF16),
        ("wo_s", [512, DIM], BF16),
        ("cs128", [128, S], BF16), ("sn128", [128, S], BF16),
    ]:
        io[name] = nc.dram_tensor(name, shape, d, kind="ExternalInput").ap()
    out = nc.dram_tensor("out", [S, DIM], BF16, kind="ExternalOutput").ap()

    with tile.TileContext(nc) as tc:
        _body(tc, io, out, lam)
    nc.compile()
    return nc


def _body(tc, io, out, lam):
    nc = tc.nc
    with ExitStack() as top:
        persist = top.enter_context(tc.tile_pool(name="persist", bufs=1))
        QT = persist.tile([128, 4, S], BF16)
        KT = persist.tile([128, 2, S], BF16)
        Vn = persist.tile([128, 16, 256], BF16)

        # B pools first (bottom of the pool stacks) so the A pools can
        # be released mid-B in LIFO order
        bctx = ExitStack()
        b = _stage_bc_pools(tc, bctx)
        actx = ExitStack()
        apsum = ExitStack()
        a = _stage_a_setup(tc, actx, apsum, io)
        # A emission: kv+q of half 0, kv of half 1. q(half 1) is woven
        # into the first B steps (B only needs K/V complete + Q half 0).
        _emit_xh(tc, a, 0)
        _emit_k(tc, a, 0, KT)
        _emit_v(tc, a, 0, Vn)
        _emit_xh(tc, a, 1)
        _emit_q(tc, a, 0, QT, KT, Vn, act_evac=True)
        a["xh"] = a["xh_next"]
        _emit_k(tc, a, 1, KT)
        # free A's x buffer and wide psum pools; the woven q(1)
        # projection streams x chunks and gets a single-buffer psum pool
        apsum.close()
        b["pss"] = bctx.enter_context(tc.tile_pool(name="pss", bufs=2,
                                                   space="PSUM"))
        b["ppvp"] = bctx.enter_context(tc.tile_pool(name="ppv", bufs=1,
                                                    space="PSUM"))
        a["ps"] = actx.enter_context(tc.tile_pool(name="pq1", bufs=1,
                                                  space="PSUM"))
        with bctx:
            _stage_bc(tc, io, out, QT, KT, Vn, lam, a, actx, b)


def _stage_a_setup(tc, ctx, psctx, io):
    nc = tc.nc
    a = {}
    a["wp"] = ctx.enter_context(tc.tile_pool(name="wqk", bufs=2))
    a["trig"] = ctx.enter_context(tc.tile_pool(name="trig", bufs=1))
    a["tmp"] = ctx.enter_context(tc.tile_pool(name="ropetmp", bufs=1))
    a["xp"] = ctx.enter_context(tc.tile_pool(name="xh", bufs=2))
    a["wvp"] = ctx.enter_context(tc.tile_pool(name="wvp", bufs=1))
    a["ps"] = psctx.enter_context(tc.tile_pool(name="ps_qk", bufs=2,
                                               space="PSUM"))
    a["psv"] = psctx.enter_context(tc.tile_pool(name="ps_v", bufs=2,
                                                space="PSUM"))
    a["xT3"] = io["xT"].rearrange("(a p) s -> p a s", p=128)
    a["wq3"] = io["wq_s"].rearrange("(a p) c -> p a c", p=128)
    a["wk3"] = io["wk_s"].rearrange("(a p) c -> p a c", p=128)

    # first x half chunked so the first matmuls can start early
    xh0 = a["xp"].tile([128, 16, 1024], BF16, tag="xh", name="xh0")
    nc.sync.dma_start(xh0[:, 0:2, :], a["xT3"][:, 0:2, 0:1024])
    wk00 = a["wp"].tile([128, 16, 128], BF16, tag="w", name="wk00")
    nc.sync.dma_start(wk00[:, 0:4, :], a["wk3"][:, 0:4, 0:128])
    nc.sync.dma_start(xh0[:, 2:4, :], a["xT3"][:, 2:4, 0:1024])
    nc.sync.dma_start(wk00[:, 4:16, :], a["wk3"][:, 4:16, 0:128])
    for c in range(1, 4):
        nc.sync.dma_start(xh0[:, c * 4:(c + 1) * 4, :],
                          a["xT3"][:, c * 4:(c + 1) * 4, 0:1024])
    a["xh0"] = xh0
    a["wk00"] = wk00

    cs = a["trig"].tile([128, S], BF16)
    sn = a["trig"].tile([128, S], BF16)
    nc.sync.dma_start(cs[:], io["cs128"][:])
    nc.sync.dma_start(sn[:], io["sn128"][:])
    a["cs"], a["sn"] = cs, sn
    wv_all = a["wvp"].tile([128, 16, 256], BF16)
    nc.sync.dma_start(wv_all[:],
                      io["wv_s"].rearrange("(a p) c -> p a c", p=128))
    a["wv_all"] = wv_all
    return a


def _emit_xh(tc, a, sq):
    if sq == 0:
        a["xh"] = a["xh0"]
        return
    nc = tc.nc
    xh = a["xp"].tile([128, 16, 1024], BF16, tag="xh", name=f"xh{sq}")
    nc.sync.dma_start(xh[:], a["xT3"][:, :, sq * 1024:(sq + 1) * 1024])
    a["xh_next"] = xh


def _rope(tc, a, pq, dest_ap, ssl, act_evac):
    """c0 = evac(pq); dest = c0*cs + shuffle(c0*sn)."""
    nc = tc.nc
    W = 1024
    c0 = a["tmp"].tile([128, W], BF16, tag="c0", name="c0")
    if act_evac:
        nc.scalar.activation(c0[:], pq[:], AF.Copy, bias=0.0, scale=1.0)
    else:
        # during the B overlap the Act engine is saturated with exps
        nc.vector.tensor_copy(c0[:], pq[:])
    t2 = a["tmp"].tile([128, W], BF16, tag="t2", name="t2")
    t2s = a["tmp"].tile([128, W], BF16, tag="t2s", name="t2s")
    nc.gpsimd.tensor_mul(t2[:], c0[:], a["sn"][:, ssl])
    nc.vector.stream_shuffle(t2s[:], t2[:], _SWAP16)
    nc.gpsimd.tensor_mul(c0[:], c0[:], a["cs"][:, ssl])
    nc.vector.tensor_add(dest_ap, c0[:], t2s[:])


def _emit_k(tc, a, sq, KT):
    """K projection (2 kv tiles) + rope for s-half sq."""
    nc = tc.nc
    W = 1024
    ssl = slice(sq * W, sq * W + W)
    xh = a["xh"]
    for kt_i in range(2):
        if sq == 0 and kt_i == 0:
            wct = a["wk00"]
        else:
            wct = a["wp"].tile([128, 16, 128], BF16, tag="w",
                               name=f"wk{sq}{kt_i}")
            nc.sync.dma_start(wct[:],
                              a["wk3"][:, :, kt_i * 128:(kt_i + 1) * 128])
        pq = a["ps"].tile([128, W], F32, tag="psqk", name="pqk")
        for dt_i in range(16):
            for nch in range(2):
                nsl = slice(nch * 512, (nch + 1) * 512)
                nc.tensor.matmul(pq[:, nsl], lhsT=wct[:, dt_i, :],
                                 rhs=xh[:, dt_i, nsl],
                                 start=(dt_i == 0), stop=(dt_i == 15))
        _rope(tc, a, pq, KT[:, kt_i, ssl], ssl, act_evac=True)


def _v_st_tasks(tc, a, sq, st, Vn, inline):
    """Closures for one V s-tile. In the woven (sq=1) path the psum is a
    256-column sub-slice of the shared single-buffer projection tile."""
    nc = tc.nc
    xh = a["xh"]
    state = {}

    def mms(lo, hi):
        def run():
            if lo == 0:
                if inline:
                    pv = a["psv"].tile([128, 256], F32, tag="psv",
                                       name="psv")
                else:
                    pvt = a["ps"].tile([128, 1024], F32, tag="psqk",
                                       name="psv1")
                    pv = pvt[:, 0:256]
                state["pv"] = pv
            for dt_i in range(lo, hi):
                nc.tensor.matmul(state["pv"],
                                 lhsT=xh[:, dt_i, st * 128:(st + 1) * 128],
                                 rhs=a["wv_all"][:, dt_i, :],
                                 start=(dt_i == 0), stop=(dt_i == 15))
        return run

    def copy():
        nc.vector.tensor_copy(Vn[:, sq * 8 + st, :], state["pv"])

    return [mms(0, 8), mms(8, 16), copy]


def _emit_v(tc, a, sq, Vn):
    for st in range(8):
        for task in _v_st_tasks(tc, a, sq, st, Vn, inline=True):
            task()


def _emit_q(tc, a, sq, QT, KT, Vn, act_evac):
    for ct in range(4):
        for task in _q_ct_tasks(tc, a, sq, ct, QT, act_evac):
            task()


def _q_ct_tasks(tc, a, sq, ct, QT, act_evac):
    """Closures for one Q head-tile projection: 16 matmul steps + rope.

    When a["xh"] is None (the B-overlap weave), each matmul streams its
    own x chunk from DRAM through a small rotating pool."""
    nc = tc.nc
    W = 1024
    ssl = slice(sq * W, sq * W + W)
    state = {}

    def load():
        wct = a["wp"].tile([128, 16, 128], BF16, tag="w",
                           name=f"wq{sq}{ct}")
        dma = nc.sync.dma_start if sq == 0 else nc.gpsimd.dma_start
        dma(wct[:], a["wq3"][:, :, ct * 128:(ct + 1) * 128])
        pq = a["ps"].tile([128, W], F32, tag="psqk", name="pqq")
        state["wct"], state["pq"] = wct, pq

    def mm(dt_i):
        def run():
            xsrc = a["xh"][:, dt_i, :]
            for nch in range(2):
                nsl = slice(nch * 512, (nch + 1) * 512)
                nc.tensor.matmul(state["pq"][:, nsl],
                                 lhsT=state["wct"][:, dt_i, :],
                                 rhs=xsrc[:, nsl],
                                 start=(dt_i == 0), stop=(dt_i == 15))
        return run

    def rope():
        _rope(tc, a, state["pq"], QT[:, ct, ssl], ssl, act_evac)

    return [load] + [mm(i) for i in range(16)] + [rope]


def _stage_bc_pools(tc, ctx):
    nc = tc.nc
    b = {"ctx": ctx}
    b["etpA"] = ctx.enter_context(tc.tile_pool(name="etpA", bufs=1))
    b["etpB"] = ctx.enter_context(tc.tile_pool(name="etpB", bufs=1))
    b["etTp"] = ctx.enter_context(tc.tile_pool(name="etTp", bufs=2))
    b["dcp"] = ctx.enter_context(tc.tile_pool(name="dcp", bufs=4))
    b["colp"] = ctx.enter_context(tc.tile_pool(name="colp", bufs=3))
    b["up"] = ctx.enter_context(tc.tile_pool(name="up", bufs=2))
    b["atp"] = ctx.enter_context(tc.tile_pool(name="atp", bufs=2))
    b["atTp"] = ctx.enter_context(tc.tile_pool(name="atTp", bufs=3))
    magp = ctx.enter_context(tc.tile_pool(name="magic", bufs=1))
    b["magicT"] = magp.tile([128, 4], mybir.dt.uint32, name="magicT")
    nc.gpsimd.memset(b["magicT"][:], 0x5F3759DF)
    return b


def _stage_bc(tc, io, out, QT, KT, Vn, lam, a, actx, b):
    """Attention + norm + wo, software-pipelined over 16 query tiles.

    step s: scores+exp+transpose(qt=s) | pv+norm(qt=s-2) | wo+out(qt=s-3).
    The three PE workloads interleave at (h, j)-beat granularity so the
    in-order PE queue always has ready work while Act runs the exps.
    The Q projection of s-half 1 is woven into steps 0-2 (its PSUM use
    borrows the banks the wo stage does not need until step 3).
    """
    nc = tc.nc
    # v+q of s-half 1, interleaved into the early-step beats; each q
    # weight load is hoisted one head-tile ahead of its matmuls
    proj = []
    for st in range(8):
        proj.extend(_v_st_tasks(tc, a, 1, st, Vn, inline=False))
    ct_tasks = [_q_ct_tasks(tc, a, 1, ct, QT, act_evac=False)
                for ct in range(4)]
    proj += [ct_tasks[0][0], ct_tasks[1][0]] + ct_tasks[0][1:]
    for ct in range(1, 4):
        if ct + 1 < 4:
            proj.append(ct_tasks[ct + 1][0])
        proj.extend(ct_tasks[ct][1:])
    proj_i = [0]

    def drain_proj(n):
        k = 0
        while k < n and proj_i[0] < len(proj):
            proj[proj_i[0]]()
            proj_i[0] += 1
            k += 1

    ctx = b["ctx"]
    etTp, dcp, colp = b["etTp"], b["dcp"], b["colp"]
    up, atp, atTp = b["up"], b["atp"], b["atTp"]
    pss, ppvp = b["pss"], b["ppvp"]
    magicT = b["magicT"]
    outwp = None
    psop = None
    if True:

        etT_t = [None] * NQT
        dc_t = [None] * NQT
        atT_t = [None] * NQT

        for step in range(NQT + 2):
            fr = step if step < NQT else None
            mid = step - 2 if 2 <= step < NQT + 2 else None
            bk = step - 3 if 3 <= step else None

            if step == 3:
                # q(half 1) projection is done; its A pools (and PSUM
                # banks) make room for the wo accumulators
                assert proj_i[0] == len(proj)
                actx.close()
                psop = ctx.enter_context(tc.tile_pool(name="pso", bufs=2,
                                                      space="PSUM"))
                outwp = ctx.enter_context(tc.tile_pool(name="outwp", bufs=2))

                wotp = ctx.enter_context(tc.tile_pool(name="wotp", bufs=1))
                wot = wotp.tile([128, 4, S], BF16, name="wot")
                nc.gpsimd.dma_start(wot[:], io["wo_s"].rearrange(
                    "(a p) c -> p a c", p=128))

            if fr is not None:
                etpool = [b["etpA"], b["etpB"]][fr % 2]
                et = etpool.tile([128, 8, 1024], BF16, tag="et", name="et")
                etT = etTp.tile([128, 64, 128], BF16, tag="etT")
                etT_t[fr] = etT
                dc = dcp.tile([128, 8], F32, tag="dc")
                dc_t[fr] = dc
            if mid is not None:
                dcm = dc_t[mid]
                rec = colp.tile([128, 4], F32, tag="rec")
                rrl = colp.tile([128, 4], F32, tag="rrl")
                msum = colp.tile([128, 4], F32, tag="msum")
                tcol = colp.tile([128, 4], F32, tag="tcol")
                ya = colp.tile([128, 4], F32, tag="ya")
                aa = colp.tile([128, 4], F32, tag="aa")
                shu = colp.tile([128, 4], mybir.dt.uint32, tag="shu")
                nc.vector.reciprocal(rec[:], dcm[:, 4:8])
                nc.vector.scalar_tensor_tensor(
                    rrl[:], dcm[:, 0:4], float(lam), rec[:],
                    op0=ALU.mult, op1=ALU.mult)
                # two psum tiles (h0+h1 / h2+h3, slots [j0, j1] per h);
                # pv beats alternate tiles (h order 0,2,1,3) so a pv
                # write never WAR-waits on the previous head's u' reads
                ppvA = ppvp.tile([128, 4, 128], F32, tag="ppvA")
                ppvB = ppvp.tile([128, 4, 128], F32, tag="ppvB")
                def ppv_ap(h, j):
                    t = ppvA if h < 2 else ppvB
                    return t[:, (h % 2) * 2 + j, :]
                u = up.tile([128, 4, 128], BF16, tag="u")
                usq = up.tile([128, 128], BF16, tag="usq")
                v2 = up.tile([128, 4, 128], BF16, tag="v2")
                at = atp.tile([128, 4, 128], BF16, tag="at")
            if bk is not None:
                atTb = atT_t[bk]
                outw = outwp.tile([128, 2048], BF16, tag="outw")

            H_ORDER = (0, 2, 1, 3)
            for beat in range(8):
                if step < 3:
                    drain_proj(5)
                # frontend: one (h, j) scores pair + exp(+denominator)
                if fr is not None:
                    hj = beat
                    j, h = divmod(hj, 4)
                    kvl, rho = h // 2, h % 2
                    jsl = slice(j * 64, j * 64 + 64)
                    qsl = slice(fr * 128, fr * 128 + 128)
                    psc = pss.tile([128, 1024], F32, tag="sc")
                    for nch in range(2):
                        nsl = slice(nch * 512, (nch + 1) * 512)
                        nc.tensor.matmul(
                            psc[:, nsl],
                            lhsT=QT[jsl, h, qsl],
                            rhs=KT[jsl, kvl,
                                   rho * 1024 + nch * 512:
                                   rho * 1024 + nch * 512 + 512],
                            start=True, stop=True)
                    nc.scalar.activation(et[:, hj, :], psc[:], AF.Exp,
                                         bias=0.0, scale=float(SCALE),
                                         accum_out=dc[:, hj:hj + 1])
                # middle: pv accumulation for one (h, j)
                if mid is not None:
                    h = H_ORDER[beat // 2]
                    j = beat % 2
                    hj = j * 4 + h
                    kvl, rho = h // 2, h % 2
                    etTm = etT_t[mid]
                    pdst = ppv_ap(h, j)
                    for kt in range(8):
                        nc.tensor.matmul(
                            pdst,
                            lhsT=etTm[:, hj * 8 + kt, :],
                            rhs=Vn[:, rho * 8 + kt,
                                   kvl * 128:(kvl + 1) * 128],
                            start=(kt == 0), stop=(kt == 7))
                    # u' for head h once both branches are accumulated
                    if j == 1:
                        nc.vector.tensor_scalar_mul(
                            v2[:, h, :], ppv_ap(h, 1), rrl[:, h:h + 1])
                        nc.vector.tensor_sub(u[:, h, :], v2[:, h, :],
                                             ppv_ap(h, 0))
                        nc.vector.scalar_tensor_tensor(
                            usq[:], u[:, h, :], 1.0, u[:, h, :],
                            op0=ALU.mult, op1=ALU.mult,
                            accum_out=msum[:, h:h + 1])
                # backend: one e-chunk of the wo matmul
                if bk is not None and beat < 4:
                    ech = beat
                    pso = psop.tile([128, 512], F32, tag="pso")
                    for r in range(4):
                        nc.tensor.matmul(
                            pso[:], lhsT=atTb[:, r, :],
                            rhs=wot[:, r, ech * 512:(ech + 1) * 512],
                            start=(r == 0), stop=(r == 3))
                    nc.vector.tensor_copy(outw[:, ech * 512:(ech + 1) * 512],
                                          pso[:])

            if step < 3:
                drain_proj(len(proj))
            if mid is not None:
                # rs = rsqrt(eps*d1^2 + sum(u^2)/128), Quake bit-trick +
                # two Newton steps on the DVE (keeps Act on {Copy, Exp}
                # only -> a single activation-table set, no reloads)
                nc.vector.scalar_tensor_tensor(
                    tcol[:], dcm[:, 0:4], float(EPS), dcm[:, 0:4],
                    op0=ALU.mult, op1=ALU.mult)
                nc.vector.scalar_tensor_tensor(
                    tcol[:], msum[:], float(1.0 / 128.0), tcol[:],
                    op0=ALU.mult, op1=ALU.add)
                nc.vector.tensor_scalar(shu[:], tcol[:].bitcast(
                    mybir.dt.uint32), 1, None, op0=ALU.arith_shift_right)
                nc.vector.tensor_sub(ya[:].bitcast(mybir.dt.uint32),
                                     magicT[:], shu[:])
                for _ in range(2):
                    nc.vector.tensor_mul(aa[:], ya[:], ya[:])
                    nc.vector.tensor_mul(aa[:], aa[:], tcol[:])
                    nc.vector.tensor_scalar(aa[:], aa[:], -0.5, 1.5,
                                            op0=ALU.mult, op1=ALU.add)
                    nc.vector.tensor_mul(ya[:], ya[:], aa[:])
                for h in range(4):
                    nc.vector.tensor_scalar_mul(at[:, h, :], u[:, h, :],
                                                ya[:, h:h + 1])
                atT = atTp.tile([128, 4, 128], BF16, tag="atT")
                atT_t[mid] = atT
                nc.sync.dma_start_transpose(atT[:], at[:])

            if fr is not None:
                nc.sync.dma_start_transpose(etT[:], et[:])

            if bk is not None:
                # Pool-issued (SWDGE) so the out write never queues behind
                # the transposes on the SP sequencer
                nc.gpsimd.dma_start(out[bk * 128:(bk + 1) * 128, :], outw[:])

        # compressed drain: the last qtile's wo follows immediately instead
        # of occupying its own pipeline step
        bk = NQT - 1
        atTb = atT_t[bk]
        outw = outwp.tile([128, 2048], BF16, tag="outw")
        for ech in range(4):
            pso = psop.tile([128, 512], F32, tag="pso")
            for r in range(4):
                nc.tensor.matmul(
                    pso[:], lhsT=atTb[:, r, :],
                    rhs=wot[:, r, ech * 512:(ech + 1) * 512],
                    start=(r == 0), stop=(r == 3))
            nc.vector.tensor_copy(outw[:, ech * 512:(ech + 1) * 512],
                                  pso[:])
        nc.gpsimd.dma_start(out[bk * 128:(bk + 1) * 128, :], outw[:])


# ---------------------------------------------------------------- host side

# row (q*32 + c*16 + i) within a branch <-> head-dim 2*(16q+i)+c
_PERM64 = np.empty(64, np.int64)
for _q in range(2):
    for _c in range(2):
        for _i in range(16):
            _PERM64[_q * 32 + _c * 16 + _i] = 2 * (16 * _q + _i) + _c


def make_core_inputs(core, x, wq, wk, wv, wo, subln_w, lambda_init,
                     freqs_cos, freqs_sin):
    b, g = divmod(core, 4)
    npdt = ml_dtypes.bfloat16
    qcols = np.empty(512, np.int64)
    for hl in range(4):
        for j in range(2):
            qcols[hl * 128 + j * 64:hl * 128 + j * 64 + 64] = \
                ((4 * g + hl) * 2 + j) * 64 + _PERM64
    kcols = np.empty(256, np.int64)
    for kvl in range(2):
        for j in range(2):
            kcols[kvl * 128 + j * 64:kvl * 128 + j * 64 + 64] = \
                ((2 * g + kvl) * 2 + j) * 64 + _PERM64
    vcols = np.arange(256) + 2 * g * 128

    cosT = np.ascontiguousarray(freqs_cos.T.astype(np.float32))  # [32, S]
    sinT = np.ascontiguousarray(freqs_sin.T.astype(np.float32))
    cs64 = np.concatenate([cosT[0:16], cosT[0:16],
                           cosT[16:32], cosT[16:32]], axis=0)
    sn64 = np.concatenate([sinT[0:16], -sinT[0:16],
                           sinT[16:32], -sinT[16:32]], axis=0)
    # wo rows carry subln*(1-lambda_init) and the global sign flip (u' = -u)
    wo_s = wo[512 * g: 512 * g + 512, :].astype(np.float32).copy()
    wo_s *= -np.tile(subln_w.astype(np.float32)
                     * (1.0 - np.float32(np.asarray(lambda_init)[0])),
                     4)[:, None]
    return {
        "xT": np.ascontiguousarray(x[b].T.astype(np.float32)).astype(npdt),
        "wq_s": np.ascontiguousarray(wq[:, qcols].astype(np.float32)).astype(npdt),
        "wk_s": np.ascontiguousarray(wk[:, kcols].astype(np.float32)).astype(npdt),
        "wv_s": np.ascontiguousarray(wv[:, vcols].astype(np.float32)).astype(npdt),
        "wo_s": wo_s.astype(npdt),
        "cs128": np.tile(cs64, (2, 1)).astype(npdt),
        "sn128": np.tile(sn64, (2, 1)).astype(npdt),
    }


def compute_lambda(lambda_q1, lambda_k1, lambda_q2, lambda_k2, lambda_init):
    l1 = np.exp(np.sum(np.float32(lambda_q1) * np.float32(lambda_k1),
                       dtype=np.float32))
    l2 = np.exp(np.sum(np.float32(lambda_q2) * np.float32(lambda_k2),
                       dtype=np.float32))
    return float(l1 - l2 + np.float32(np.asarray(lambda_init)[0]))


def kernel(x, wq, wk, wv, wo, lambda_q1, lambda_k1, lambda_q2, lambda_k2,
           lambda_init, subln_w, freqs_cos, freqs_sin):
    global LAST_RESULTS
    x = np.asarray(x); wq = np.asarray(wq); wk = np.asarray(wk)
    wv = np.asarray(wv); wo = np.asarray(wo)
    lam = compute_lambda(lambda_q1, lambda_k1, lambda_q2, lambda_k2, lambda_init)

    nc = build_program(lam)
    in_maps = [make_core_inputs(c, x, wq, wk, wv, wo,
                                np.asarray(subln_w), np.asarray(lambda_init),
                                np.asarray(freqs_cos), np.asarray(freqs_sin))
               for c in range(NCORES)]
    res = run_bass_kernel_spmd(nc, in_maps, list(range(NCORES)), trace=TRACE)
    LAST_RESULTS = res
    outs = [res.results[c]["out"] for c in range(NCORES)]
    full = np.empty((B, S, DIM), np.float32)
    for b in range(B):
        full[b] = (outs[4 * b].astype(np.float32)
                   + outs[4 * b + 1].astype(np.float32)
                   + outs[4 * b + 2].astype(np.float32)
                   + outs[4 * b + 3].astype(np.float32))
    return full
